# revision 37
# baseline (speedup 1.0000x reference)
"""Trainium2 Bass kernel for CellPathwayPoolingAggregator (segment mean).

out[b, p] = (1/segment_sizes[p]) * sum_{k: segment_ids[k]==p} x[b, flat_indices[k]]

Strategy (8 cores, sharded by contiguous pathway ranges):
  - Host: split the 1000 pathways into 8 contiguous ranges (<=128 pathways
    each) balancing per-core unique-gene counts. For each core, dedupe its
    gene rows and pack them into contiguous DRAM slabs in float8_e3m4
    (T k-tiles of 128 gene rows, in 2-tile DMA groups laid out so each DMA
    is a perfect 128-partition x 4KB-per-partition contiguous transfer).
    A per-core count matrix S (e3m4, exact for small counts) carries the
    (gene, pathway) multiplicities.
  - Device (per core): memset-fed warmup matmuls ramp the PE p-state while
    plain sequential dma_starts pull the slabs into SBUF; a PE matmul per
    (k-tile, 512-batch bank) accumulates pathway x batch sums into 4 PSUM
    banks (S tile stationary, gathered rows moving, fp32 accumulate).
    DVE/ACT scale rows by 1/segment_sizes into bf16; two stores on separate
    HWDGE queues write the (128, 2048) output slice; host reassembles.

e3m4 quantization of x gives rel err ~1.3e-2 (< 2e-2 tolerance); counts and
1/size scaling stay exact (counts are small ints; scale applied in f32).
"""

import sys

import numpy as np
import ml_dtypes

_TRN_REPO = "/opt/trn_rl_repo"
if _TRN_REPO not in sys.path:
    sys.path.insert(0, _TRN_REPO)

import concourse.bass as bass  # noqa: F401
import concourse.mybir as mybir
import concourse.tile as tile
from concourse import bacc
from concourse.bass_utils import run_bass_kernel_spmd

B, G, P = 2048, 10000, 1000
NCORES = 8
PC = 128          # max pathways per core (psum partition dim)
NB = B // 512     # matmul N-slices per K-tile (4 banks of 512 f32)
NWARM = 12        # PE warmup matmuls (ramp the tensor engine p-state)
NDR = 3           # trailing 2-tile groups processed as e4m3 DoubleRow pairs

F8 = ml_dtypes.float8_e3m4
F8DR = ml_dtypes.float8_e4m3


def _group_sizes(T):
    """k-tiles per DMA group: uniform 2-tile (4KB/partition) groups.
    (A smaller leading group starves the PE at t=1 and resets its p-state
    ramp — measured worse.)"""
    gs = [2] * (T // 2)
    if T % 2:
        gs.append(1)
    return gs


def _split_ranges(seg_sorted, idx_sorted):
    """Contiguous pathway ranges, <=128 pathways each, minimizing the max
    per-core count of UNIQUE genes (which sets T and hence DMA/PE work)."""
    seg_starts = np.searchsorted(seg_sorted, np.arange(P + 1), side="left")

    def feasible(U):
        bounds = [0]
        for c in range(NCORES):
            lo_p = bounds[-1]
            if lo_p >= P:
                return None
            best = lo_p + 1
            hi_cap = min(P, lo_p + PC)
            lo_e = seg_starts[lo_p]
            for hi_p in range(lo_p + 1, hi_cap + 1):
                nu = len(np.unique(idx_sorted[lo_e : seg_starts[hi_p]]))
                if nu <= U:
                    best = hi_p
                else:
                    break
            bounds.append(best)
        return bounds if bounds[-1] >= P else None

    lo_t, hi_t = 1, (len(idx_sorted) + 127) // 128 + 1
    best_bounds = None
    while lo_t <= hi_t:
        mid = (lo_t + hi_t) // 2
        b = feasible(mid * 128)
        if b is not None:
            best_bounds = b
            hi_t = mid - 1
        else:
            lo_t = mid + 1
    if best_bounds is None:
        best_bounds = list(
            np.minimum(np.arange(NCORES + 1) * ((P + NCORES - 1) // NCORES), P)
        )
    best_bounds[-1] = P
    return best_bounds


def _build_schedule(flat_indices, segment_ids):
    seg = np.asarray(segment_ids, dtype=np.int64)
    idx = np.asarray(flat_indices, dtype=np.int64)
    order = np.argsort(seg, kind="stable")
    seg = seg[order]
    idx = idx[order]

    bounds = _split_ranges(seg, idx)
    cores = []
    for c in range(NCORES):
        lo_p, hi_p = bounds[c], bounds[c + 1]
        lo = np.searchsorted(seg, lo_p, side="left")
        hi = np.searchsorted(seg, hi_p, side="left")
        uidx, inv = np.unique(idx[lo:hi], return_inverse=True)
        cores.append((lo_p, hi_p, uidx, inv, seg[lo:hi] - lo_p))

    T = max(1, max((len(u) + 127) // 128 for _, _, u, _, _ in cores))
    T += T % 2  # even T: uniform 2-tile DMA groups (single-tag tile pool)
    Kpad = T * 128

    s_sbs, uidx_pads = [], []
    for lo_p, hi_p, uidx, inv, cols in cores:
        nu = len(uidx)
        # padded unique-gene list; pad rows point at gene 0 but S is zero there
        uidx_pad = np.concatenate([uidx, np.zeros(Kpad - nu, np.int64)])
        S = np.zeros((Kpad, PC), np.float32)
        np.add.at(S, (inv, cols), 1.0)
        S = S.astype(F8)
        s_sbs.append(
            np.ascontiguousarray(
                S.reshape(T, 128, PC).transpose(1, 0, 2).reshape(128, -1)
            )
        )
        uidx_pads.append(uidx_pad)
    return bounds, uidx_pads, s_sbs, T


def _build_program(T):
    nc = bacc.Bacc(
        "TRN2",
        target_bir_lowering=False,
        debug=False,
        num_devices=NCORES,
        num_swdge_queues=1,
    )
    f8, f32, bf16 = mybir.dt.float8e3, mybir.dt.float32, mybir.dt.bfloat16
    f8dr = mybir.dt.float8e4

    gsz = _group_sizes(T)
    NG = len(gsz)
    ndr = min(NDR, max(0, NG - 2))
    dr0 = NG - ndr  # first DoubleRow group index
    slab_ds = [
        nc.dram_tensor(
            f"slab{g}",
            [128, 2, B] if g >= dr0 else [128, gsz[g] * B],
            f8dr if g >= dr0 else f8,
            kind="ExternalInput",
        )
        for g in range(NG)
    ]
    Tn = int(sum(gsz[:dr0]))  # k-tiles handled by normal e3m4 matmuls
    s_d = nc.dram_tensor("smat", [128, Tn * PC], f8, kind="ExternalInput")
    sdr_d = (
        nc.dram_tensor("smatdr", [128, 2 * ndr, PC], f8dr, kind="ExternalInput")
        if ndr
        else None
    )
    inv_d = nc.dram_tensor("invsz", [128, 1], f32, kind="ExternalInput")
    out_d = nc.dram_tensor("out", [PC, B], bf16, kind="ExternalOutput")

    with tile.TileContext(nc) as tc:
        with (
            tc.tile_pool(name="sb", bufs=1) as pool,
            tc.tile_pool(name="slabp", bufs=NG) as gpool,
            tc.tile_pool(name="psum", bufs=1, space="PSUM") as ppool,
        ):
            # Warmup source: memset on the (otherwise idle) Vector engine —
            # no DMA dependency, so the tensor engine starts ramping its
            # p-state immediately.
            wsrc = pool.tile([128, 512], f8, tag="wsrc")
            nc.vector.memset(wsrc[:], 0)

            # smat/invsz on the Scalar HWDGE queue, in parallel with the
            # slab groups on Sync. (GpSimd DMA is SWDGE — too slow here.)
            s_sb = pool.tile([128, Tn * PC], f8, tag="smat")
            nc.scalar.dma_start(s_sb[:], s_d.ap())
            if ndr:
                sdr_sb = pool.tile([128, 2 * ndr, PC], f8dr, tag="smatdr")
                nc.scalar.dma_start(sdr_sb[:], sdr_d.ap())
            inv_sb = pool.tile([128, 1], f32, tag="invsz")
            nc.scalar.dma_start(inv_sb[:], inv_d.ap())

            psb = [
                ppool.tile([128, 512], f32, tag=f"ps{n}", name=f"ps{n}")
                for n in range(NB)
            ]
            wps = ppool.tile([128, 512], f32, tag="pswarm", name="pswarm")

            gts = []
            for g in range(NG):
                if g >= dr0:
                    gt = gpool.tile([128, 2, B], f8dr, tag="gtdr")
                else:
                    gt = gpool.tile([128, gsz[g] * B], f8, tag="gt")
                nc.sync.dma_start(gt[:], slab_ds[g].ap())
                gts.append(gt)

            # Warmup matmuls: ramp the PE p-state while the first slab
            # group + smat are still in flight.
            for _ in range(NWARM):
                nc.tensor.matmul(
                    wps[:], wsrc[:, :128], wsrc[:], start=True, stop=True
                )

            tstarts = np.cumsum([0] + gsz)
            for g in range(dr0):
                gt = gts[g]
                for cc in range(gsz[g]):
                    t = int(tstarts[g]) + cc
                    for n in range(NB):
                        nc.tensor.matmul(
                            psb[n][:],
                            s_sb[:, t * PC : (t + 1) * PC],
                            gt[:, cc * B + n * 512 : cc * B + (n + 1) * 512],
                            start=(t == 0),
                            stop=(t == T - 1 and not ndr),
                        )
            # Trailing groups as e4m3 DoubleRow pairs: each matmul contracts
            # both k-tiles of the pair at 0.5 cycles/row.
            for j in range(ndr):
                gt = gts[dr0 + j]
                for n in range(NB):
                    nc.tensor.matmul(
                        psb[n][:],
                        sdr_sb[:, 2 * j : 2 * j + 2, :],
                        gt[:, :, n * 512 : (n + 1) * 512],
                        start=False,
                        stop=(j == ndr - 1),
                        perf_mode=mybir.MatmulPerfMode.DoubleRow,
                    )

            # Eviction: one bf16 tile per bank (DVE even banks, ACT odd) so
            # each bank's store departs right after its own eviction; four
            # stores alternate Sync/Scalar HWDGE queues to overlap issue
            # cost, pulling the last DMA tick (which the exit barrier's
            # completion wait keys off) earlier.
            for n in range(NB):
                ot = pool.tile([128, 512], bf16, tag=f"ot{n}", name=f"ot{n}")
                if n % 2 == 1:
                    nc.scalar.activation(
                        ot[:],
                        psb[n][:],
                        mybir.ActivationFunctionType.Identity,
                        scale=inv_sb[:],
                    )
                else:
                    nc.vector.tensor_scalar_mul(ot[:], psb[n][:], inv_sb[:])
                eng = nc.sync if n % 2 == 0 else nc.scalar
                eng.dma_start(
                    out_d.ap()[:, n * 512 : (n + 1) * 512], ot[:]
                )
    return nc


def _prepare(gene_set_features, flat_indices, segment_ids, segment_sizes):
    bounds, uidx_pads, s_sbs, T = _build_schedule(flat_indices, segment_ids)
    nc = _build_program(T)
    nc.compile()

    x = np.asarray(gene_set_features, dtype=np.float32)
    xtf = np.ascontiguousarray(x.T)             # (G, B) f32
    xt8 = xtf.astype(F8)                        # e3m4 for normal tiles
    sizes = np.asarray(segment_sizes, dtype=np.float32)
    gsz = _group_sizes(T)
    NG = len(gsz)
    ndr = min(NDR, max(0, NG - 2))
    dr0 = NG - ndr
    Tn = int(sum(gsz[:dr0]))
    tstarts = np.cumsum([0] + gsz)

    in_maps = []
    for c in range(NCORES):
        lo_p, hi_p = bounds[c], bounds[c + 1]
        inv = np.ones((128, 1), np.float32)
        inv[: hi_p - lo_p, 0] = 1.0 / sizes[lo_p:hi_p]
        m = {"invsz": inv, "smat": np.ascontiguousarray(s_sbs[c][:, : Tn * PC])}
        if ndr:
            # counts are small ints — e3m4 -> e4m3 recast is exact
            m["smatdr"] = np.ascontiguousarray(
                s_sbs[c][:, Tn * PC :].astype(F8DR).reshape(128, 2 * ndr, PC)
            )
        up = uidx_pads[c].reshape(T, 128)  # [t, p]
        for g, gs in enumerate(gsz):
            # slab row p holds the gs gene rows for partition p of group g,
            # concatenated: [ktile tstarts[g]+cc, partition p] for cc in gs.
            tiles = up[tstarts[g] : tstarts[g] + gs]        # [gs, 128]
            perm = tiles.T.reshape(-1)                       # [p, cc]
            if g >= dr0:
                m[f"slab{g}"] = np.ascontiguousarray(
                    xtf[perm].astype(F8DR).reshape(128, 2, B)
                )
            else:
                m[f"slab{g}"] = np.ascontiguousarray(
                    xt8[perm].reshape(128, gs * B)
                )
        in_maps.append(m)
    return nc, in_maps, bounds


def kernel(gene_set_features, flat_indices, segment_ids, segment_sizes, _res_hook=None):
    nc, in_maps, bounds = _prepare(
        gene_set_features, flat_indices, segment_ids, segment_sizes
    )
    res = run_bass_kernel_spmd(nc, in_maps, list(range(NCORES)))
    if _res_hook is not None:
        _res_hook(res)
    outT = np.empty((P, B), np.float32)
    for c in range(NCORES):
        lo_p, hi_p = bounds[c], bounds[c + 1]
        outT[lo_p:hi_p] = np.asarray(res.results[c]["out"]).astype(np.float32)[
            : hi_p - lo_p
        ]
    return np.ascontiguousarray(outT.T)
